# revision 1
# baseline (speedup 1.0000x reference)
"""Trainium2 Bass kernel for nn_BDH_1726576853700 (sparse_attention).

3-layer sparse-attention net: B=1, T=1024, D=256, NH=4, N=8192, VOCAB=256.

Sharding over 8 NeuronCores: device d -> (head h=d//2, half=d%2) — each device
owns a 4096-wide slice of one head's sparse latent dim, permuted evens-first so
the RoPE pair partner of latent tile i is tile i+16.

v2 rewrite (vs the phase-serial baseline):
  - Phase A runs m-tiles in pair-completing order (0,16,1,17,...) in dense
    4-m matmul bursts; all relus on ACT; rope is emitted per 2-pair group
    with [P,2,T]-wide DVE/Pool ops so it overlaps the projection matmuls.
  - Scores run in fp8e4 + DoubleRow (2 k-tiles contracted per step; the
    2^QR_EXP qr scale rides on the host cos/sin tables and is removed at
    S copy-out).  Accuracy impact measured at ~1e-3 end-to-end — attention
    scores feed a scale-invariant LN so fp8 is safe here (the enc/encv/dec
    projections are NOT; they stay fp16).
  - Two-pass K-split accumulation: G1 chunks (7 PSUM banks) accumulate the
    early-roped DoubleRow steps while late pairs are still roping, then
    finish; G2 chunks + the deferred ykv rows recycle freed banks against
    SBUF-held S (packed strict-upper layout + masked diag tiles).
  - Phase D uses [P,512] PSUM half-tiles with relu*x_sparse fused into
    scalar_tensor_tensor on DVE or split relu(ACT)+mul(DVE/Pool).
  - LayerNorm chains are emitted stage-major in half-batches of 4 t-tiles
    (no head-of-line blocking), fp16 end-to-end.
  - DMAs: flat 2D slab APs (HWDGE path); compute-dependent DMAs ride the
    gpsimd/ACT queues so their data-waits never block the SP prefetch
    stream.
Collectives (3 pair + 3 all-reduce) are unchanged from the baseline.

PSUM discipline: every accumulation group owns a full bank ([P,512] f32
tiles even when only half is used) — start=True clears the whole bank.
Engine legality learned the hard way: GPSIMD (Pool) cannot read PSUM and
cannot run TensorScalarPtr — it only gets SBUF-to-SBUF tensor_tensor ops.
"""

import math
import sys

for _p in ("/opt/trn_rl_repo",):
    if _p not in sys.path:
        sys.path.insert(0, _p)

import numpy as np

import concourse.bass as bass
import concourse.mybir as mybir
import concourse.tile as tile
from concourse import bacc, bass_utils
from concourse.masks import make_identity

# ---- problem constants (hardcoded per contract) ----
B, T, D, NH, N = 1, 1024, 256, 4, 8192
VOCAB = 256
N_LAYER = 3
EPS = 1e-5
TWO_PI = 2.0 * math.pi
N_CORES = 8
NLOC = N // 2          # latent columns per device: 4096
P = 128
NT = T // P            # 8 t-tiles
KD = D // P            # 2 d-tiles
NM = NLOC // P         # 32 n-tiles per device
NPAIR = NM // 2        # 16 rope pairs
HDT = mybir.dt.float16
F32 = mybir.dt.float32
F8 = mybir.dt.float8e4
YKV_SCALE = 1.0 / 256.0
# scores run in fp8e4 DoubleRow; qr is scaled by 2^QR_EXP via the host
# cos/sin tables and S is descaled by 2^-2*QR_EXP at PSUM copy-out
QR_EXP = 6
S_DESCALE = 2.0 ** (-2 * QR_EXP)

# phase-A emission order: pair-completing (0,16,1,17,...)
EMIT = [(e // 2) + NPAIR * (e % 2) for e in range(NM)]
# scores chunks (c, i): rows t-tile i, cols [base, 512(c+1))
CHUNKS = [(c, i) for c in range(2) for i in range(4 * c + 4)]
G1 = [(0, 0), (0, 1), (0, 2), (0, 3), (1, 0), (1, 1), (1, 2), (1, 3)]
G2 = [(1, 4), (1, 5), (1, 6), (1, 7)]
KSPLIT = 10            # rope pairs in K1
# DoubleRow step r contracts k-tiles (2r, 2r+1); step r (r<8) needs the
# even-part qr of pairs 2r,2r+1, step r+8 their odd parts.  Pair-completion
# order within each split:
R1 = [0, 8, 1, 9, 2, 10, 3, 11, 4, 12, 5, 13]  # ready once pairs 0..11 roped
R2 = [6, 14, 7, 15]                            # ready once pairs 12..15 roped
# packed S_sb layout: row i stores global cols [128(i+1), 1024)
SOFF = [0]
for _i in range(NT):
    SOFF.append(SOFF[-1] + (T - P * (_i + 1)))  # total 3584

_CACHE = {}


def _build_program(dbg=False, use_collectives=True, n_layers=N_LAYER):
    def emit_allreduce(nc, groups, ins, outs, halves=None):
        if use_collectives:
            nc.gpsimd.collective_compute(
                "AllReduce", mybir.AluOpType.add, replica_groups=groups,
                ins=ins, outs=outs)
        else:
            # sim stand-in: off the SP queue (no head-of-line blocking of
            # the prefetch stream), split in two so the first back-DMA can
            # start as soon as its half is through
            if halves is None:
                nc.scalar.dma_start(outs[0], ins[0])
            else:
                for (o, i) in zip(halves(outs[0]), halves(ins[0])):
                    nc.scalar.dma_start(o, i)

    nc = bacc.Bacc("TRN2", target_bir_lowering=False, debug=False,
                   num_devices=N_CORES)

    x0_d = nc.dram_tensor("x0", [T, D], HDT, kind="ExternalInput")
    # host slabs, see _host_inputs for layouts
    encw_d = nc.dram_tensor("encw", [P, NM * KD * P], HDT, kind="ExternalInput")
    encvw_d = nc.dram_tensor("encvw", [P, NM * KD * P], HDT,
                             kind="ExternalInput")
    decw_d = nc.dram_tensor("decw", [P, NM * D], HDT, kind="ExternalInput")
    ct_d = nc.dram_tensor("ct", [P, NPAIR * T], HDT, kind="ExternalInput")
    st_d = nc.dram_tensor("st", [P, NPAIR * T], HDT, kind="ExternalInput")
    lmh_d = nc.dram_tensor("lmh", [P, KD * VOCAB], HDT, kind="ExternalInput")
    umask_d = nc.dram_tensor("umask", [P, P], F32, kind="ExternalInput")
    logits_d = nc.dram_tensor("logits", [T, VOCAB], F32, kind="ExternalOutput")

    PAIR_GROUPS = [[0, 1], [2, 3], [4, 5], [6, 7]]
    ALL_GROUP = [list(range(N_CORES))]

    with tile.TileContext(nc) as tc:
        persist = tc.alloc_tile_pool(name="persist", bufs=1)
        dram = tc.alloc_tile_pool(name="dram", bufs=1, space="DRAM")

        # persistent SBUF state
        x_h = persist.tile([P, NT, D], HDT)         # residual stream (natural)
        xT_h = persist.tile([P, KD, T], HDT)        # x^T
        ykvT_h = persist.tile([P, KD, T], HDT)      # ykv_ln^T
        x_sp = persist.tile([P, NM, T], HDT)        # x_sparse^T
        qr8 = persist.tile([P, NPAIR, 2, T], F8)    # roped, DR-interleaved
        S_sb = persist.tile([P, SOFF[NT]], HDT)     # packed strict-upper S
        sdiag = persist.tile([P, NT, P], HDT)       # masked diagonal blocks
        ykv_pre = persist.tile([P, NT, D], HDT)
        ykv_post = persist.tile([P, NT, D], HDT)
        ymlpT_pre = persist.tile([P, KD, T], HDT)
        ymlpT_post = persist.tile([P, KD, T], HDT)
        lmh_sb = persist.tile([P, KD, VOCAB], HDT)
        umask_sb = persist.tile([P, P], F32)
        ident = persist.tile([P, P], HDT)
        eps_sb = persist.tile([P, 1], F32)

        x0_sb = persist.tile([P, NT, D], HDT)
        nc.sync.dma_start(
            x0_sb[:], x0_d.ap().rearrange("(j p) d -> p j d", p=P))
        nc.vector.memset(eps_sb[:], float(EPS))
        nc.sync.dma_start(umask_sb[:], umask_d.ap())
        make_identity(nc, ident[:])
        nc.sync.dma_start(
            lmh_sb[:], lmh_d.ap().rearrange("p (k v) -> p k v", k=KD))

        # streaming / working pools
        wenc = tc.alloc_tile_pool(name="wenc", bufs=3)      # enc/encv slabs
        wdec = tc.alloc_tile_pool(name="wdec", bufs=2)
        csp = tc.alloc_tile_pool(name="csp", bufs=2)        # ct/st tiles
        ropep = tc.alloc_tile_pool(name="ropep", bufs=2)
        yxp = tc.alloc_tile_pool(name="yxp", bufs=3)
        lnp = tc.alloc_tile_pool(name="lnp", bufs=4)
        statp = tc.alloc_tile_pool(name="statp", bufs=10)

        engines = {"dve": nc.vector, "act": nc.scalar, "pool": nc.gpsimd}

        def copy_from_psum(eng, dst_ap, src_ap):
            if eng == "act":
                nc.scalar.copy(out=dst_ap, in_=src_ap)
            elif eng == "pool":
                nc.gpsimd.tensor_copy(out=dst_ap, in_=src_ap)
            else:
                nc.vector.tensor_copy(out=dst_ap, in_=src_ap)

        def relu_psum(eng, dst_ap, src_ap):
            if eng == "act":
                nc.scalar.activation(out=dst_ap, in_=src_ap,
                                     func=mybir.ActivationFunctionType.Relu)
            else:
                engines[eng].tensor_scalar_max(dst_ap, src_ap, 0.0)

        def ln_batch(pairs):
            """Stage-major LayerNorm over a batch of (src_ap, out_ap) [P, D]
            tiles — every stage's ops are emitted together so independent
            chains pipeline instead of head-of-line blocking engine queues."""
            n = len(pairs)
            stats = [statp.tile([P, 6], F32, name="ln_stats") for _ in range(n)]
            mvs = [statp.tile([P, 2], F32, name="ln_mv") for _ in range(n)]
            rstds = [statp.tile([P, 1], F32, name="ln_rstd") for _ in range(n)]
            for q, (src, _) in enumerate(pairs):
                nc.vector.bn_stats(out=stats[q][:], in_=src)
            for q in range(n):
                nc.vector.bn_aggr(out=mvs[q][:], in_=stats[q][:])
            for q in range(n):
                nc.scalar.activation(out=rstds[q][:], in_=mvs[q][:, 1:2],
                                     func=mybir.ActivationFunctionType.Sqrt,
                                     bias=eps_sb[:])
            for q in range(n):
                nc.vector.reciprocal(out=rstds[q][:], in_=rstds[q][:])
            for q, (src, out) in enumerate(pairs):
                nc.vector.tensor_scalar(out=out, in0=src,
                                        scalar1=mvs[q][:, 0:1],
                                        scalar2=rstds[q][:],
                                        op0=mybir.AluOpType.subtract,
                                        op1=mybir.AluOpType.mult)

        def layer_norm(src_ap, out_ap):
            ln_batch([(src_ap, out_ap)])

        _tp_cycle = ["dve", "act"]
        _tp_idx = [0]

        def transpose_into(dst_ap, src_ap, pst_pool):
            """PE-transpose a [P, P] fp16 SBUF block into dst (via PSUM)."""
            pst = pst_pool.tile([P, P], HDT, name="pst")
            nc.tensor.transpose(pst[:], src_ap, ident[:])
            eng = _tp_cycle[_tp_idx[0] % 2]
            _tp_idx[0] += 1
            copy_from_psum(eng, dst_ap, pst[:])

        def set_xT_from(j, pst_pool):
            for k in range(KD):
                transpose_into(xT_h[:, k, j * P:(j + 1) * P],
                               x_h[:, j, k * P:(k + 1) * P], pst_pool)

        # ---- initial x = ln(embed[idx]) (gather done on host into x0) ----
        with tc.tile_pool(name="ps_init", bufs=4, space="PSUM") as ps_init:
            for hb in range(2):
                js = range(4 * hb, 4 * hb + 4)
                ln_batch([(x0_sb[:, j, :], x_h[:, j, :]) for j in js])
                for j in js:
                    set_xT_from(j, ps_init)

        # ---- layers ----
        for layer in range(n_layers):
            # === Phase A: x_sparse^T = relu(enc^T x^T), rope -> qr ===
            with tc.tile_pool(name=f"psA_{layer}", bufs=4,
                              space="PSUM") as psA:
                for e4 in range(0, NM, 4):
                    # 4-m-tile group (= one 2-pair rope group): weight DMA,
                    # then a dense 16-matmul burst, then the 4 relus, then
                    # rope — bursts keep the PE p-state warm.
                    et = wenc.tile([P, 4 * KD * P], HDT, name="enc_t", bufs=2)
                    o0 = e4 * KD * P
                    nc.sync.dma_start(
                        et[:], encw_d.ap()[:, o0:o0 + 4 * KD * P])
                    pair0 = e4 // 2
                    ctt = csp.tile([P, 2, T], HDT, name="ctt", tag="ctt")
                    stt = csp.tile([P, 2, T], HDT, name="stt", tag="stt")
                    nc.sync.dma_start(
                        ctt[:], ct_d.ap()[:, pair0 * T:(pair0 + 2) * T]
                        .rearrange("p (i t) -> p i t", t=T))
                    nc.sync.dma_start(
                        stt[:], st_d.ap()[:, pair0 * T:(pair0 + 2) * T]
                        .rearrange("p (i t) -> p i t", t=T))
                    pss = []
                    for sub in range(4):
                        ps = psA.tile([P, T], F32, name="psA")
                        for c in range(2):
                            for k in range(KD):
                                nc.tensor.matmul(
                                    ps[:, c * 512:(c + 1) * 512],
                                    lhsT=et[:, (sub * KD + k) * P:
                                             (sub * KD + k + 1) * P],
                                    rhs=xT_h[:, k, c * 512:(c + 1) * 512],
                                    start=(k == 0), stop=(k == KD - 1))
                        pss.append(ps)
                    for sub in range(4):
                        relu_psum("act", x_sp[:, EMIT[e4 + sub], :],
                                  pss[sub][:])
                    for _rope_once in range(1):
                        pair = pair0 + 1
                        if True:
                            # pairs (2g, 2g+1) complete -> rope both at once
                            # with [P,2,T]-wide ops into fp8 qr8; Pool takes a
                            # share via plain tensor_tensor (stt is not legal
                            # on Pool).
                            g = pair // 2
                            xe2 = x_sp[:, 2 * g:2 * g + 2, :]
                            xo2 = x_sp[:, 2 * g + NPAIR:2 * g + NPAIR + 2, :]
                            qe2 = qr8[:, g, :, :]
                            qo2 = qr8[:, g + 8, :, :]
                            # Pool only runs muls that depend on nothing but
                            # relu outputs, so its queue never stalls; all
                            # combines stay on DVE.  bufs=6 keeps >1 group in
                            # flight to hide the cross-engine chain latency.
                            t2 = ropep.tile([P, 2, T], HDT, name="rope_t2",
                                            tag="rope_t", bufs=7)
                            nc.gpsimd.tensor_mul(t2[:], xo2, stt[:])
                            t4 = ropep.tile([P, 2, T], HDT, name="rope_t4",
                                            tag="rope_t", bufs=7)
                            if g % 2 == 1:
                                nc.gpsimd.tensor_mul(t4[:], xe2, stt[:])
                            else:
                                nc.vector.tensor_mul(t4[:], xe2, stt[:])
                            t1 = ropep.tile([P, 2, T], HDT, name="rope_t1",
                                            tag="rope_t", bufs=7)
                            nc.vector.tensor_mul(t1[:], xe2, ctt[:])
                            t3 = ropep.tile([P, 2, T], HDT, name="rope_t3",
                                            tag="rope_t", bufs=7)
                            nc.vector.tensor_mul(t3[:], xo2, ctt[:])
                            nc.vector.tensor_sub(qe2, t1[:], t2[:])
                            nc.vector.tensor_add(qo2, t3[:], t4[:])

            # === Phase B: scores (2-pass K-split) + deferred ykv ===
            with tc.tile_pool(name=f"psB_{layer}", bufs=8,
                              space="PSUM") as psB:
                def s_chunk_mm(ps, c, i, rs, start, stop):
                    base = max(c * 512, i * P)
                    width = (c + 1) * 512 - base
                    for n, r_ in enumerate(rs):
                        nc.tensor.matmul(
                            ps[:, :width],
                            lhsT=qr8[:, r_, :, i * P:(i + 1) * P],
                            rhs=qr8[:, r_, :, base:base + width],
                            start=(start and n == 0),
                            stop=(stop and n == len(rs) - 1),
                            perf_mode=mybir.MatmulPerfMode.DoubleRow)

                def s_chunk_out(ps, c, i, idx):
                    base = max(c * 512, i * P)
                    width = (c + 1) * 512 - base
                    diag = (c == i // 4)
                    skip = P if diag else 0  # diag block goes to sdiag only
                    w2 = width - skip
                    if w2 > 0:
                        dst0 = SOFF[i] + (base + skip) - P * (i + 1)
                        if idx % 2 == 0:
                            nc.scalar.mul(out=S_sb[:, dst0:dst0 + w2],
                                          in_=ps[:, skip:width],
                                          mul=S_DESCALE)
                        else:
                            nc.vector.tensor_scalar_mul(
                                S_sb[:, dst0:dst0 + w2],
                                ps[:, skip:width], S_DESCALE)
                    if diag:
                        nc.vector.tensor_mul(sdiag[:, i, :], ps[:, 0:P],
                                             umask_sb[:])

                def ykv_row(j):
                    ykv_ps = psB.tile([P, 512], F32, name=f"ykv_ps{j}", tag="psB")
                    for i2 in range(j + 1):
                        if i2 == j:
                            lhsT = sdiag[:, i2, :]
                        else:
                            o = SOFF[i2] + P * (j - i2 - 1)
                            lhsT = S_sb[:, o:o + P]
                        nc.tensor.matmul(
                            ykv_ps[:, :D], lhsT=lhsT, rhs=x_h[:, i2, :],
                            start=(i2 == 0), stop=(i2 == j))
                    nc.scalar.mul(out=ykv_pre[:, j, :], in_=ykv_ps[:, :D],
                                  mul=YKV_SCALE)

                g1_ps = []
                for (c, i) in G1:
                    ps = psB.tile([P, 512], F32, name=f"psS_{c}_{i}", tag="psB")
                    s_chunk_mm(ps, c, i, R1, start=True, stop=False)
                    g1_ps.append(ps)
                for idx, (c, i) in enumerate(G1):
                    s_chunk_mm(g1_ps[idx], c, i, R2, start=False, stop=True)
                    s_chunk_out(g1_ps[idx], c, i, idx)
                    if c == 0:
                        # row i only needs chunks (0, i2<=i) + its diagonal,
                        # all already out -> front-load the first AR half
                        ykv_row(i)
                for idx, (c, i) in enumerate(G2):
                    ps = psB.tile([P, 512], F32, name=f"psS_{c}_{i}", tag="psB")
                    s_chunk_mm(ps, c, i, R1 + R2, start=True, stop=True)
                    s_chunk_out(ps, c, i, idx)
                    if i >= 4:
                        # row i's last dependency is its own diag chunk
                        ykv_row(i)

            # === Phase C: pair AllReduce of ykv, layernorm, transpose ===
            ar_in = dram.tile([T, D], HDT, name=f"arin_{layer}",
                              tag=f"arin_{layer}")
            ar_out = dram.tile([T, D], HDT, name=f"arout_{layer}",
                               tag=f"arout_{layer}")
            for h in range(2):
                nc.gpsimd.dma_start(
                    ar_in.rearrange("(j p) d -> p j d", p=P)[:, 4 * h:4 * h + 4],
                    ykv_pre[:, 4 * h:4 * h + 4, :])
            emit_allreduce(nc, PAIR_GROUPS, [ar_in.opt()], [ar_out.opt()],
                           halves=lambda a: [a[0:T // 2, :], a[T // 2:, :]])
            with tc.tile_pool(name=f"psT_{layer}", bufs=4,
                              space="PSUM") as psT:
                for hb in range(2):
                    js = list(range(4 * hb, 4 * hb + 4))
                    nc.gpsimd.dma_start(
                        ykv_post[:, js[0]:js[0] + 4, :],
                        ar_out.rearrange("(j p) d -> p j d", p=P)[
                            :, js[0]:js[0] + 4])
                    yls = [lnp.tile([P, D], HDT, name="ykv_ln", bufs=4)
                           for _ in js]
                    ln_batch([(ykv_post[:, j, :], yls[q][:])
                              for q, j in enumerate(js)])
                    for q, j in enumerate(js):
                        for k in range(KD):
                            transpose_into(ykvT_h[:, k, j * P:(j + 1) * P],
                                           yls[q][:, k * P:(k + 1) * P], psT)

            # === Phase D: y_sp = relu(encv^T ykv^T); xy; ymlp^T accum ===
            ar2_in = dram.tile([D, T], HDT, name=f"ar2in_{layer}",
                               tag=f"ar2in_{layer}")
            with tc.tile_pool(name=f"psD_{layer}", bufs=4,
                              space="PSUM") as psD, \
                 tc.tile_pool(name=f"psM_{layer}", bufs=1,
                              space="PSUM") as psM:
                ymlpT_ps = [psM.tile([P, T], F32, name=f"ymlpT_ps{k}",
                                     tag=f"ymlpT_ps{k}") for k in range(KD)]
                for e in range(0, NM, 2):
                    et = wenc.tile([P, 2 * KD * P], HDT, name="encv_t",
                                   bufs=2)
                    o0 = e * KD * P
                    nc.sync.dma_start(
                        et[:], encvw_d.ap()[:, o0:o0 + 2 * KD * P])
                    dwt = wdec.tile([P, 2 * D], HDT, name="dec_t")
                    nc.sync.dma_start(
                        dwt[:], decw_d.ap()[:, e * D:(e + 2) * D])
                    for sub in range(2):
                        m = e + sub
                        xy = yxp.tile([P, T], HDT, name="xy")
                        for c in range(2):
                            ps = psD.tile([P, 512], F32, name="psD")
                            for k in range(KD):
                                nc.tensor.matmul(
                                    ps[:],
                                    lhsT=et[:, (sub * KD + k) * P:
                                             (sub * KD + k + 1) * P],
                                    rhs=ykvT_h[:, k, c * 512:(c + 1) * 512],
                                    start=(k == 0), stop=(k == KD - 1))
                            xyh = xy[:, c * 512:(c + 1) * 512]
                            xsh = x_sp[:, m, c * 512:(c + 1) * 512]
                            nc.vector.scalar_tensor_tensor(
                                xyh, ps[:], 0.0, xsh,
                                mybir.AluOpType.max, mybir.AluOpType.mult)
                            for k in range(KD):
                                nc.tensor.matmul(
                                    ymlpT_ps[k][:, c * 512:(c + 1) * 512],
                                    lhsT=dwt[:, (sub * KD + k) * P:
                                             (sub * KD + k + 1) * P],
                                    rhs=xyh,
                                    start=(m == 0), stop=(m == NM - 1))
                for h in range(2):
                    half = slice(512 * h, 512 * h + 512)
                    for k in range(KD):
                        if k == 0:
                            nc.scalar.copy(out=ymlpT_pre[:, k, half],
                                           in_=ymlpT_ps[k][:, half])
                        else:
                            nc.vector.tensor_copy(
                                out=ymlpT_pre[:, k, half],
                                in_=ymlpT_ps[k][:, half])
                    nc.gpsimd.dma_start(
                        ar2_in.rearrange("(k p) t -> p k t", p=P)[:, :, half],
                        ymlpT_pre[:, :, half])

            # === Phase E: 8-way AllReduce of ymlp^T; x = ln(x + ln(ymlp)) ===
            ar2_out = dram.tile(
                [D, T], HDT, name=f"ar2out_{layer}",
                tag=f"ar2out_{layer}",
                addr_space="Shared" if use_collectives else "Local")
            emit_allreduce(nc, ALL_GROUP, [ar2_in.opt()], [ar2_out.opt()],
                           halves=lambda a: [a[:, 0:T // 2], a[:, T // 2:]])
            with tc.tile_pool(name=f"psE_{layer}", bufs=4,
                              space="PSUM") as psE, \
                 tc.tile_pool(name=f"psL_{layer}", bufs=2,
                              space="PSUM") as psL:
                for hb in range(2):
                    js = list(range(4 * hb, 4 * hb + 4))
                    # t-half back-DMA: cols for these 4 j-tiles
                    nc.gpsimd.dma_start(
                        ymlpT_post[:, :, 512 * hb:512 * hb + 512],
                        ar2_out.rearrange("(k p) t -> p k t", p=P)[
                            :, :, 512 * hb:512 * hb + 512])
                    ymts = [lnp.tile([P, D], HDT, name="ymt", bufs=4)
                            for _ in js]
                    for q, j in enumerate(js):
                        for k in range(KD):
                            transpose_into(ymts[q][:, k * P:(k + 1) * P],
                                           ymlpT_post[:, k,
                                                      j * P:(j + 1) * P],
                                           psE)
                    us = [lnp.tile([P, D], HDT, name="u_ln", bufs=4)
                          for _ in js]
                    ln_batch([(ymts[q][:], us[q][:])
                              for q in range(4)])
                    xns = [lnp.tile([P, D], HDT, name="xn", bufs=4)
                           for _ in js]
                    for q, j in enumerate(js):
                        eng = nc.vector if q % 2 == 0 else nc.gpsimd
                        eng.tensor_add(xns[q][:], x_h[:, j, :], us[q][:])
                    ln_batch([(xns[q][:], x_h[:, j, :])
                              for q, j in enumerate(js)])
                    for j in js:
                        set_xT_from(j, psE)
                    if layer == n_layers - 1:
                        # overlap logits with the other E half-batch
                        pss = []
                        for j in js:
                            ps = psL.tile([P, VOCAB], F32, name="psLt")
                            for k in range(KD):
                                nc.tensor.matmul(
                                    ps[:],
                                    lhsT=xT_h[:, k, j * P:(j + 1) * P],
                                    rhs=lmh_sb[:, k, :],
                                    start=(k == 0), stop=(k == KD - 1))
                            pss.append(ps)
                        lgs = []
                        for q, j in enumerate(js):
                            lg = lnp.tile([P, VOCAB], HDT, name="lgt",
                                          bufs=2)
                            nc.scalar.copy(out=lg[:], in_=pss[q][:])
                            lgs.append(lg)
                        for q, j in enumerate(js):
                            nc.gpsimd.dma_start(
                                logits_d.ap()[j * P:(j + 1) * P, :],
                                lgs[q][:])

        for _pool in (statp, lnp, yxp, ropep, csp, wdec, wenc, dram, persist):
            _pool.release()

    nc.compile()
    return nc


def _host_inputs(idx, embed, encoder, encoder_v, decoder, lm_head):
    """Build the 8 per-core input maps (host-side sharding + slab layouts)."""
    f16 = np.float16
    idx = np.asarray(idx).reshape(-1).astype(np.int64)
    embed = np.asarray(embed, np.float32)
    enc = np.asarray(encoder, np.float32)
    encv = np.asarray(encoder_v, np.float32)
    dec = np.asarray(decoder, np.float32)
    lmh = np.asarray(lm_head, np.float32)

    x0 = embed[idx]  # [T, D] gather on host (pure indexing)

    t = np.arange(0, N, dtype=np.float32)
    q = np.floor(t / 2.0) * 2.0
    freqs = (1.0 / ((2.0 ** 16) ** (q / N)) / TWO_PI).astype(np.float32)
    tvec = np.arange(T, dtype=np.float32)

    umask = (np.arange(P)[:, None] < np.arange(P)[None, :]).astype(np.float32) * S_DESCALE
    lmh_slab = np.ascontiguousarray(
        lmh.reshape(KD, P, VOCAB).transpose(1, 0, 2).reshape(P, KD * VOCAB),
        f16)

    in_maps = []
    for d in range(N_CORES):
        h, half = d // 2, d % 2
        perm = np.concatenate([np.arange(0, NLOC, 2),
                               np.arange(1, NLOC, 2)]) + half * NLOC
        f_loc = freqs[perm[:NLOC // 2]]
        ph = (tvec[None, :] * f_loc[:, None]).astype(np.float32) % 1.0

        # enc slabs [P, m, k, col] with phase-A emission order for encw
        encp = enc[h][:, perm]            # [D, NLOC]
        encvp = encv[h][:, perm]
        # [D, NLOC] -> [k, P(part-of-d), m, col] -> want [p, m, k, col]
        def enc_slab(w, order):
            wr = w.reshape(KD, P, NM, P)      # [k, p, m, col]
            wr = wr.transpose(1, 2, 0, 3)     # [p, m, k, col]
            wr = wr[:, order]
            return np.ascontiguousarray(
                wr.reshape(P, NM * KD * P), f16)

        decp = dec[h * N + perm, :]       # [NLOC, D]
        dec_slab = np.ascontiguousarray(
            decp.reshape(NM, P, D).transpose(1, 0, 2).reshape(P, NM * D), f16)

        ct = np.cos(TWO_PI * ph) * (2.0 ** QR_EXP)   # [NLOC//2, T]
        st = np.sin(TWO_PI * ph) * (2.0 ** QR_EXP)
        ct_slab = np.ascontiguousarray(
            ct.reshape(NPAIR, P, T).transpose(1, 0, 2).reshape(P, NPAIR * T),
            f16)
        st_slab = np.ascontiguousarray(
            st.reshape(NPAIR, P, T).transpose(1, 0, 2).reshape(P, NPAIR * T),
            f16)

        in_maps.append({
            "x0": np.ascontiguousarray(x0, f16),
            "encw": enc_slab(encp, EMIT),
            "encvw": enc_slab(encvp, list(range(NM))),
            "decw": dec_slab,
            "ct": ct_slab,
            "st": st_slab,
            "lmh": lmh_slab,
            "umask": umask,
        })
    return in_maps


def kernel(idx, embed, encoder, encoder_v, decoder, lm_head,
           _trace=False, _tmpdir=None):
    if "nc" not in _CACHE:
        _CACHE["nc"] = _build_program()
    nc = _CACHE["nc"]
    in_maps = _host_inputs(idx, embed, encoder, encoder_v, decoder, lm_head)
    res = bass_utils.run_bass_kernel_spmd(
        nc, in_maps, core_ids=list(range(N_CORES)),
        trace=_trace, tmpdir=_tmpdir)
    _CACHE["last_results"] = res
    logits = res.results[0]["logits"].astype(np.float32).reshape(B, T, VOCAB)
    return logits

